# revision 1
# baseline (speedup 1.0000x reference)
"""CRF log-likelihood loss kernel for Trainium2 (8 NeuronCores, Bass/Tile).

Strategy (data-parallel over batch, per sharding hint):
  - B=256 batch rows sharded 32 per core; W/b/CRF tables replicated.
  - Host pre-transposes each emissions shard to [H, T, Bs] so the device
    matmul (contract over H on partitions) needs no on-device transposes.
  - Device: logits^T[k, (t,b)] = W^T @ emisT  (PE, PSUM accumulate over 2
    h-chunks); X = exp(logits + bias) (ACT, bias fused); gold-tag logit sum
    via tensor_tensor_reduce against a host-built one-hot (DVE).
  - Forward algorithm in the linear domain: a_t[j,b] stays transposed
    [K, Bs] so each step is ONE matmul with lhsT = exp(transitions)
    augmented with a ones-column (emits column sums for free) plus ONE DVE
    multiply by X_t. Every 8 steps the state is renormalized by the sum row
    (reciprocal + ones-outer-product broadcast matmul + multiply); the norms
    are recorded and folded back in on the host.
  - Host finishes: logZ_b = sum(ln s) + ln(sum_j a_final[j,b]*exp(end_j));
    numerator = device gold-logit sum + tags-only terms (start/trans/end/bias)
    computed on host; final scalar = sum_b(score_b - logZ_b).
"""

import numpy as np

B, T, H, K = 256, 512, 256, 32
NCORES = 8
BS = B // NCORES          # 32 batch rows per core
NT = T * BS               # 16384 tokens per core
CHUNK = 2048              # tokens per DMA chunk
SUB = 512                 # tokens per matmul / X tile
NCHUNK = NT // CHUNK      # 8
NSUB = CHUNK // SUB       # 4
NXT = NT // SUB           # 32 X tiles
TS_PER_XT = SUB // BS     # 16 t-steps per X tile
RENORM = 8                # renormalize each chain's state every 8 rounds
NRENORM = 32              # slab slots per chain (bwd uses 31)
NROUND = 255              # bidirectional: fwd t=1..255, bwd t=510..256

_BUILT = {}
LAST_RESULTS = None


def _build_nc(parts="all"):
    import concourse.bacc as bacc
    import concourse.tile as tile
    from concourse import mybir
    from contextlib import ExitStack

    import concourse.bass as bass
    from concourse import bass_isa

    do_bulk = parts in ("all", "bulk", "bulk_nottr", "bulk_ttr2", "bulk_mr")
    do_ttr = parts in ("all", "bulk", "bulk_ttr2", "bulk_mr")
    # TENSOR_TENSOR_REDUCE crashes this HW/FW (NRT_EXEC_UNIT_UNRECOVERABLE,
    # verified by bisection) — use mult + reduce_sum + add instead.
    ttr_mode = {"bulk_ttr2": "ttr2", "bulk": "ttr"}.get(parts, "mr")
    do_chain = parts in ("all", "chain", "chain_norenorm")
    do_renorm = parts in ("all", "chain")

    f32 = mybir.dt.float32
    bf16 = mybir.dt.bfloat16
    Exp = mybir.ActivationFunctionType.Exp
    Copy = mybir.ActivationFunctionType.Copy
    mult = mybir.AluOpType.mult
    add = mybir.AluOpType.add

    nc = bacc.Bacc("TRN2", target_bir_lowering=False, debug=False,
                   num_devices=NCORES)

    emisT = nc.declare_dram_parameter("emisT", [2, 128, NT], f32, isOutput=False)
    oht = nc.declare_dram_parameter("oht", [K, NT], f32, isOutput=False)
    wT = nc.declare_dram_parameter("wT", [2, 128, K], f32, isOutput=False)
    ehat = nc.declare_dram_parameter("ehat", [K, K], bf16, isOutput=False)
    ebwd = nc.declare_dram_parameter("ebwd", [K, K], bf16, isOutput=False)
    bvec = nc.declare_dram_parameter("bvec", [K, 1], f32, isOutput=False)
    estart = nc.declare_dram_parameter("estart", [K, 1], f32, isOutput=False)
    eend = nc.declare_dram_parameter("eend", [K, 1], f32, isOutput=False)
    amid_d = nc.declare_dram_parameter("amid", [K, BS], f32, isOutput=True)
    vmid_d = nc.declare_dram_parameter("vmid", [K, BS], f32, isOutput=True)
    shf_d = nc.declare_dram_parameter("shist", [1, NRENORM * BS], f32, isOutput=True)
    shb_d = nc.declare_dram_parameter("shistb", [1, NRENORM * BS], f32, isOutput=True)
    gold_d = nc.declare_dram_parameter("gold", [K, 1], f32, isOutput=True)

    with ExitStack() as ctx:
        tc = ctx.enter_context(tile.TileContext(nc))
        consts = ctx.enter_context(tc.tile_pool(name="consts", bufs=1))
        emis_pool = ctx.enter_context(tc.tile_pool(name="emis", bufs=3))
        oh_pool = ctx.enter_context(tc.tile_pool(name="oh", bufs=2))
        xpool = ctx.enter_context(tc.tile_pool(name="xp", bufs=NXT))
        apool = ctx.enter_context(tc.tile_pool(name="ap", bufs=16))
        tmppool = ctx.enter_context(tc.tile_pool(name="tp", bufs=2))
        rpool = ctx.enter_context(tc.tile_pool(name="rp", bufs=2))
        bcpool = ctx.enter_context(tc.tile_pool(name="bc", bufs=3))
        scrpool = ctx.enter_context(tc.tile_pool(name="scr", bufs=2))
        psum_l = ctx.enter_context(tc.tile_pool(name="pl", bufs=4, space="PSUM"))
        psum_c = ctx.enter_context(tc.tile_pool(name="pc", bufs=2, space="PSUM"))

        # constants
        w0 = consts.tile([128, K], f32)
        w1 = consts.tile([128, K], f32)
        ehat_sb = consts.tile([K, K], bf16)
        ebwd_sb = consts.tile([K, K], bf16)
        b_sb = consts.tile([K, 1], f32)
        estart_sb = consts.tile([K, 1], f32)
        eend_sb = consts.tile([K, 1], f32)
        shf_sb = consts.tile([1, NRENORM * BS], f32)
        shb_sb = consts.tile([1, NRENORM * BS], f32)
        gacc = consts.tile([K, 1], f32)
        nc.sync.dma_start(out=w0, in_=wT[0])
        nc.sync.dma_start(out=w1, in_=wT[1])
        nc.sync.dma_start(out=ehat_sb, in_=ehat[:, :])
        nc.sync.dma_start(out=ebwd_sb, in_=ebwd[:, :])
        nc.sync.dma_start(out=b_sb, in_=bvec[:, :])
        nc.sync.dma_start(out=estart_sb, in_=estart[:, :])
        nc.sync.dma_start(out=eend_sb, in_=eend[:, :])

        nc.vector.memset(gacc, 0.0)
        nc.vector.memset(shf_sb, 1.0)
        nc.vector.memset(shb_sb, 1.0)

        # ---- bulk: logits, X = exp(logits + b), gold-tag logit sum ----
        xtiles = [None] * NXT
        nttr = 0
        chunk_order = [0, 7, 1, 6, 2, 5, 3, 4]
        for c in chunk_order:
            cs, ce = c * CHUNK, (c + 1) * CHUNK
            if do_bulk:
                e0 = emis_pool.tile([128, CHUNK], f32, tag="e0")
                e1 = emis_pool.tile([128, CHUNK], f32, tag="e1")
                nc.sync.dma_start(out=e0, in_=emisT[0, :, cs:ce])
                nc.sync.dma_start(out=e1, in_=emisT[1, :, cs:ce])
                ohc = oh_pool.tile([K, CHUNK], f32, tag="ohc")
                nc.sync.dma_start(out=ohc, in_=oht[:, cs:ce])
            for s in range(NSUB):
                xt = xpool.tile([K, SUB], f32, tag="xt")
                xtiles[c * NSUB + s] = xt
                if not do_bulk:
                    nc.vector.memset(xt, 1.0)
                    continue
                pl = psum_l.tile([K, SUB], f32, tag="pl")
                nc.tensor.matmul(pl, w0, e0[:, s * SUB:(s + 1) * SUB],
                                 start=True, stop=False)
                nc.tensor.matmul(pl, w1, e1[:, s * SUB:(s + 1) * SUB],
                                 start=False, stop=True)
                nc.scalar.activation(out=xt, in_=pl, func=Exp, bias=b_sb)
                if do_ttr:
                    scr = scrpool.tile([K, SUB], f32, tag="scr")
                    ohsl = ohc[:, s * SUB:(s + 1) * SUB]
                    if ttr_mode == "ttr":
                        init = 0.0 if nttr == 0 else gacc
                        nc.vector.tensor_tensor_reduce(
                            out=scr, in0=pl, in1=ohsl,
                            scale=1.0, scalar=init, op0=mult, op1=add,
                            accum_out=gacc)
                    elif ttr_mode == "ttr2":
                        acc_c = rpool.tile([K, 1], f32, tag="acc_c")
                        nc.vector.tensor_tensor_reduce(
                            out=scr, in0=pl, in1=ohsl,
                            scale=1.0, scalar=0.0, op0=mult, op1=add,
                            accum_out=acc_c)
                        nc.vector.tensor_add(gacc, gacc, acc_c)
                    else:
                        acc_c = rpool.tile([K, 1], f32, tag="acc_c")
                        nc.vector.tensor_mul(scr, pl, ohsl)
                        nc.vector.reduce_sum(acc_c, scr,
                                             axis=mybir.AxisListType.X)
                        nc.vector.tensor_add(gacc, gacc, acc_c)
                    nttr += 1

        # ---- bidirectional chain (bf16 states, single-pass PE matmuls):
        # forward alpha from t=0 and backward beta from t=511 run as two
        # independent 255-round recurrences that interleave on PE/DVE,
        # halving the serial latency; Z = alpha_255^T E beta-part on host ----
        def xslice(t):
            return xtiles[t // TS_PER_XT][:, (t % TS_PER_XT) * BS:
                                          (t % TS_PER_XT + 1) * BS]

        a_prev = apool.tile([K, BS], bf16, tag="af")
        nc.vector.tensor_scalar(out=a_prev, in0=xslice(0),
                                scalar1=estart_sb, scalar2=None, op0=mult)
        v_prev = apool.tile([K, BS], bf16, tag="av")
        nc.vector.tensor_scalar(out=v_prev, in0=xslice(T - 1),
                                scalar1=eend_sb, scalar2=None, op0=mult)

        if do_chain:
            # Renorm schedule, staggered so the two chains' extra DVE work
            # lands on different rounds, and spread over rounds r+2 / r+3
            # via deferred emission (the in-order DVE queue head-of-line
            # blocks on anything emitted too early).
            # chain f: measure r%8==2, divide r%8==7 (lag 5)
            # chain v: measure r%8==6, divide r%8==3 from r=11 (lag 5)
            u32 = mybir.dt.uint32
            states = {
                "f": dict(a=a_prev, lhs=ehat_sb, slab=shf_sb, q=[], nm=0,
                          pm=2, pa=7, amin=7, nmax=NRENORM),
                "v": dict(a=v_prev, lhs=ebwd_sb, slab=shb_sb, q=[], nm=0,
                          pm=6, pa=3, amin=11, nmax=NRENORM - 1),
            }
            deferred = {}
            for r in range(1, NROUND + 1):
                for job in deferred.pop(r, []):
                    job()
                for h in ("f", "v"):
                    st = states[h]
                    t = r if h == "f" else T - 1 - r
                    if (do_renorm and r % RENORM == st["pa"]
                            and r >= st["amin"] and st["q"]):
                        xsl = st["q"].pop(0)  # X slice pre-scaled by 1/s
                    else:
                        xsl = xslice(t)
                    pc = psum_c.tile([K, BS], f32, tag="pc" + h)
                    nc.tensor.matmul(pc, st["lhs"], st["a"],
                                     start=True, stop=True)
                    a_new = apool.tile([K, BS], bf16, tag="a" + h)
                    nc.vector.tensor_mul(a_new, pc, xsl)
                    st["a"] = a_new
                    if (do_renorm and r % RENORM == st["pm"]
                            and st["nm"] < st["nmax"]):
                        slot = st["nm"]
                        st["nm"] += 1
                        ta = r + 5 if h == "f" else T - 1 - (r + 5)
                        bc = bcpool.tile([K, BS], f32, tag="bc" + h)
                        nc.gpsimd.partition_all_reduce(
                            bc, st["a"], channels=K,
                            reduce_op=bass_isa.ReduceOp.add)
                        rbc = bcpool.tile([K, BS], f32, tag="rbc" + h)
                        xm = bcpool.tile([K, BS], f32, tag="xm" + h)
                        st["q"].append(xm)

                        def mk(st=st, slot=slot, ta=ta, bc=bc, rbc=rbc,
                               xm=xm):
                            def ts_job():
                                # power-of-two reciprocal: flip the f32
                                # exponent field -> r = 2^(255-e); exact
                                # to record and to multiply.
                                nc.vector.tensor_scalar(
                                    out=rbc[:, :].bitcast(u32),
                                    in0=bc[:, :].bitcast(u32),
                                    scalar1=0x7F800000, scalar2=0x7F800000,
                                    op0=mybir.AluOpType.bitwise_and,
                                    op1=mybir.AluOpType.bitwise_xor)

                            def xm_job():
                                nc.scalar.activation(
                                    out=st["slab"][0:1,
                                                   slot * BS:(slot + 1) * BS],
                                    in_=rbc[0:1, :], func=Copy)
                                nc.vector.tensor_mul(xm, xslice(ta), rbc)
                            return ts_job, xm_job

                        ts_job, xm_job = mk()
                        deferred.setdefault(r + 2, []).append(ts_job)
                        deferred.setdefault(r + 3, []).append(xm_job)
            for jobs in deferred.values():
                for job in jobs:
                    job()
            a_prev = states["f"]["a"]
            v_prev = states["v"]["a"]

        nc.gpsimd.dma_start(out=amid_d[:, :], in_=a_prev)
        nc.gpsimd.dma_start(out=vmid_d[:, :], in_=v_prev)
        nc.sync.dma_start(out=shf_d[:, :], in_=shf_sb)
        nc.sync.dma_start(out=shb_d[:, :], in_=shb_sb)
        nc.sync.dma_start(out=gold_d[:, :], in_=gacc)

    nc.compile()
    return nc


def _numpy_fallback(emissions, W, b, start_transitions, transitions,
                    end_transitions, tags, mask):
    # Exact replication of the reference semantics (used only if mask is not
    # all-ones, which the spec's input fill guarantees never happens).
    e = emissions.astype(np.float64)
    logits = e @ W.astype(np.float64) + b.astype(np.float64)
    mf = mask.astype(np.float64)
    st = start_transitions.astype(np.float64)
    tr = transitions.astype(np.float64)
    en = end_transitions.astype(np.float64)
    Bn = logits.shape[0]
    bar = np.arange(Bn)
    first = tags[:, 0]
    score = st[first] + logits[bar, 0, first]
    prev = first.copy()
    for t in range(1, T):
        tg = tags[:, t]
        stepv = tr[prev, tg] + logits[bar, t, tg]
        score = score + stepv * mf[:, t]
        prev = np.where(mf[:, t] > 0, tg, prev)
    score = score + en[prev]
    alpha = st[None, :] + logits[:, 0]
    for t in range(1, T):
        nxt = alpha[:, :, None] + tr[None, :, :]
        m = nxt.max(axis=1, keepdims=True)
        nxt = np.log(np.exp(nxt - m).sum(axis=1)) + m[:, 0, :] + logits[:, t]
        alpha = np.where(mf[:, t:t + 1] > 0, nxt, alpha)
    fin = alpha + en[None, :]
    m = fin.max(axis=1, keepdims=True)
    logz = np.log(np.exp(fin - m).sum(axis=1)) + m[:, 0]
    return np.asarray((score - logz).sum(), dtype=np.float32)


def kernel(emissions, W, b, start_transitions, transitions, end_transitions,
           tags, mask):
    global LAST_RESULTS
    emissions = np.ascontiguousarray(np.asarray(emissions, dtype=np.float32))
    W = np.asarray(W, dtype=np.float32)
    b = np.asarray(b, dtype=np.float32)
    start_transitions = np.asarray(start_transitions, dtype=np.float32)
    transitions = np.asarray(transitions, dtype=np.float32)
    end_transitions = np.asarray(end_transitions, dtype=np.float32)
    tags = np.asarray(tags).astype(np.int64)
    mask = np.asarray(mask).astype(bool)

    if not mask.all():
        return _numpy_fallback(emissions, W, b, start_transitions, transitions,
                               end_transitions, tags, mask)

    from concourse.bass_utils import run_bass_kernel_spmd

    if "nc" not in _BUILT:
        _BUILT["nc"] = _build_nc()
    nc = _BUILT["nc"]

    wT_h = np.ascontiguousarray(W.reshape(2, 128, K))
    import ml_dtypes
    E32 = np.exp(transitions).astype(np.float32)
    ehat_h = np.ascontiguousarray(E32.astype(ml_dtypes.bfloat16))
    ebwd_h = np.ascontiguousarray(E32.T.astype(ml_dtypes.bfloat16))
    bvec_h = np.ascontiguousarray(b.reshape(K, 1))
    estart_h = np.ascontiguousarray(np.exp(start_transitions)
                                    .astype(np.float32).reshape(K, 1))
    eend_h = np.ascontiguousarray(np.exp(end_transitions)
                                  .astype(np.float32).reshape(K, 1))

    in_maps = []
    for c in range(NCORES):
        sh = emissions[c * BS:(c + 1) * BS]              # [BS, T, H]
        emisT_h = np.ascontiguousarray(sh.transpose(2, 1, 0)).reshape(2, 128, NT)
        tg = tags[c * BS:(c + 1) * BS]                   # [BS, T]
        oht_h = np.ascontiguousarray(
            (np.arange(K, dtype=np.int64)[:, None, None] == tg.T[None, :, :])
            .astype(np.float32).reshape(K, NT))
        in_maps.append(dict(emisT=emisT_h, oht=oht_h, wT=wT_h, ehat=ehat_h,
                            ebwd=ebwd_h, bvec=bvec_h, estart=estart_h,
                            eend=eend_h))

    res = run_bass_kernel_spmd(nc, in_maps, list(range(NCORES)))
    LAST_RESULTS = res

    E64 = np.exp(transitions.astype(np.float64))
    total = 0.0
    for c in range(NCORES):
        out = res.results[c]
        amid = out["amid"].astype(np.float64)            # [K, BS] alpha_255
        vmid = out["vmid"].astype(np.float64)            # [K, BS] x*beta_256
        shf = out["shist"].astype(np.float64).reshape(NRENORM, BS)
        shb = out["shistb"].astype(np.float64).reshape(NRENORM, BS)
        gold = out["gold"].astype(np.float64)            # [K, 1]
        # Z_b = alpha_255^T E (x_256*beta_256), scaled by recorded norms
        zmid = np.einsum("kb,kj,jb->b", amid, E64, vmid)
        logz = -np.log(shf).sum(axis=0) - np.log(shb).sum(axis=0) + np.log(zmid)
        tg = tags[c * BS:(c + 1) * BS]
        hterm = (start_transitions.astype(np.float64)[tg[:, 0]].sum()
                 + transitions.astype(np.float64)[tg[:, :-1], tg[:, 1:]].sum()
                 + end_transitions.astype(np.float64)[tg[:, -1]].sum()
                 + b.astype(np.float64)[tg].sum())
        total += gold.sum() + hterm - logz.sum()

    return np.asarray(total, dtype=np.float32)



# revision 5
# speedup vs baseline: 1.1997x; 1.1997x over previous
"""CRF log-likelihood loss kernel for Trainium2 (8 NeuronCores, Bass/Tile).

Strategy (data-parallel over batch, per sharding hint):
  - B=256 batch rows sharded 32 per core; W/b/CRF tables replicated.
  - The forward algorithm runs bidirectionally in the linear domain:
    fwd alpha from t=0 and bwd beta from t=511 meet at t=255/256 and are
    stitched on host (Z = alpha^T E (x*beta)).  Both chains are packed
    into ONE state tile s[64, BS] (fwd rows 0:32, bwd rows 32:64) so each
    round is a single [64,64] block-diag matmul (PE) plus a single
    elementwise multiply by the paired X slice (DVE).
  - X tiles are produced as [64, 512] (fwd tokens asc on rows 0:32, bwd
    tokens desc on rows 32:64) by zero-padded double-wide W matmuls over
    bf16 emissions, PSUM-accumulated over 4 passes, then exp via ACT with
    the bias fused.  A constant prescale exp(-PRE) per step is folded into
    the bias so the chain needs NO renormalization (bf16 exponent range
    absorbs the drift); host adds T*PRE back to logZ exactly.
  - Bulk X production is emission-interleaved into the chain's PE idle
    gaps (1 bulk matmul per 4 chain rounds) so it costs ~no wall time.
  - Host finishes: numerator (gold-path score incl. emissions term) via
    numpy, logZ_b = log(amid^T E vmid) + T*PRE; result = sum(score-logZ).
"""

import numpy as np

B, T, H, K = 256, 512, 256, 32
NCORES = 8
BS = B // NCORES          # 32 batch rows per core
NT = T * BS               # 16384 tokens per core
SUB = 512                 # tokens per X tile half (16 t-steps * 32 b)
NXT = 16                  # X tiles (each covers 16 fwd + 16 bwd t-steps)
TS_PER_XT = SUB // BS     # 16 t-steps per X tile half
NROUND = 255              # fwd t=1..255, bwd t=510..256
PRE = 5.75 * 0.6931471805599453  # per-step prescale (log-domain)

_BUILT = {}
LAST_RESULTS = None


def _build_nc():
    import concourse.bacc as bacc
    import concourse.tile as tile
    from concourse import mybir
    from contextlib import ExitStack

    f32 = mybir.dt.float32
    bf16 = mybir.dt.bfloat16
    Exp = mybir.ActivationFunctionType.Exp
    mult = mybir.AluOpType.mult

    nc = bacc.Bacc("TRN2", target_bir_lowering=False, debug=False,
                   num_devices=NCORES)

    # emisF: fwd tokens t=0..255 asc; emisB: bwd tokens t=511..256 desc.
    # Each [2 (h-half), 128, 8192] bf16, token order (t, b) b-minor.
    emisF = nc.declare_dram_parameter("emisF", [2, 128, NXT * SUB], bf16,
                                      isOutput=False)
    emisB = nc.declare_dram_parameter("emisB", [2, 128, NXT * SUB], bf16,
                                      isOutput=False)
    # wlo: cols 0:32 = W[h,:], cols 32:64 = 0; whi: mirrored. Per h-half.
    wpack = nc.declare_dram_parameter("wpack", [4, 128, 2 * K], bf16,
                                      isOutput=False)
    eblk = nc.declare_dram_parameter("eblk", [2 * K, 2 * K], bf16,
                                     isOutput=False)
    bvec = nc.declare_dram_parameter("bvec", [2 * K, 1], f32, isOutput=False)
    svec = nc.declare_dram_parameter("svec", [2 * K, 1], f32, isOutput=False)
    smid_d = nc.declare_dram_parameter("smid", [2 * K, BS], bf16,
                                       isOutput=True)

    with ExitStack() as ctx:
        tc = ctx.enter_context(tile.TileContext(nc))
        consts = ctx.enter_context(tc.tile_pool(name="consts", bufs=1))
        emis_pool = ctx.enter_context(tc.tile_pool(name="emis", bufs=12))
        xpool = ctx.enter_context(tc.tile_pool(name="xp", bufs=NXT))
        apool = ctx.enter_context(tc.tile_pool(name="ap", bufs=8))
        psum_l = ctx.enter_context(tc.tile_pool(name="pl", bufs=3,
                                                space="PSUM"))
        psum_c = ctx.enter_context(tc.tile_pool(name="pc", bufs=4,
                                                space="PSUM"))

        # constants
        wsb = [consts.tile([128, 2 * K], bf16, name=f"w{i}", tag=f"w{i}")
               for i in range(4)]
        eblk_sb = consts.tile([2 * K, 2 * K], bf16)
        b_sb = consts.tile([2 * K, 1], f32)
        s_sb = consts.tile([2 * K, 1], f32)
        for i in range(4):
            nc.sync.dma_start(out=wsb[i], in_=wpack[i])
        nc.sync.dma_start(out=eblk_sb, in_=eblk[:, :])
        nc.sync.dma_start(out=b_sb, in_=bvec[:, :])
        nc.sync.dma_start(out=s_sb, in_=svec[:, :])

        xtiles = [None] * NXT
        emis_tiles = {}

        def emit_dma(i):
            # 4 slabs for X tile i: (F h0, F h1, B h0, B h1)
            sl = []
            for src in (emisF, emisB):
                for h in range(2):
                    e = emis_pool.tile([128, SUB], bf16, tag="e")
                    nc.sync.dma_start(
                        out=e, in_=src[h, :, i * SUB:(i + 1) * SUB])
                    sl.append(e)
            emis_tiles[i] = sl

        def emit_mm(i, j):
            # j-th of 4 PSUM-accumulated matmuls for X tile i
            # order: (wlo,F h0), (wlo',F h1), (whi,B h0), (whi',B h1)
            sl = emis_tiles[i]
            rhs = sl[(0, 1, 2, 3)[j]]
            w = wsb[(0, 1, 2, 3)[j]]
            if j == 0:
                emis_tiles[i + 100] = psum_l.tile([2 * K, SUB], f32,
                                                  name="plx", tag="plx")
            pl = emis_tiles[i + 100]
            nc.tensor.matmul(pl, w, rhs, start=(j == 0), stop=(j == 3))

        def emit_act(i):
            xt = xpool.tile([2 * K, SUB], f32, tag="xt")
            xtiles[i] = xt
            nc.scalar.activation(out=xt, in_=emis_tiles[i + 100],
                                 func=Exp, bias=b_sb)
            del emis_tiles[i + 100]
            del emis_tiles[i]

        def xsl(r):
            # X slice for chain round r: fwd t=r rows 0:32, bwd t=511-r
            # rows 32:64 (bwd tokens stored t-desc so same columns).
            return xtiles[r // TS_PER_XT][
                :, (r % TS_PER_XT) * BS:(r % TS_PER_XT + 1) * BS]

        # prologue: tiles 0 and 1 fully, DMA for 2..3
        for i in (0, 1):
            emit_dma(i)
        for i in (0, 1):
            for j in range(4):
                emit_mm(i, j)
            emit_act(i)
        emit_dma(2)
        emit_dma(3)

        # chain init: s0 = X(col 0 of tile 0) * [estart; eend]
        s_prev = apool.tile([2 * K, BS], bf16, tag="s")
        nc.vector.tensor_scalar(out=s_prev, in0=xsl(0),
                                scalar1=s_sb, scalar2=None, op0=mult)

        # chain rounds, with bulk work for tile (r//16)+2 interleaved
        # one matmul per 4 rounds; DMA 2 tiles ahead of that.
        for r in range(1, NROUND + 1):
            ri = r % TS_PER_XT
            ti = r // TS_PER_XT + 2
            if ti < NXT:
                if ri == 1:
                    emit_dma(ti + 2) if ti + 2 < NXT else None
                if ri in (2, 6, 10, 14):
                    emit_mm(ti, ri // 4)
                if ri == 15:
                    emit_act(ti)
            pc = psum_c.tile([2 * K, BS], f32, tag="pc")
            nc.tensor.matmul(pc, eblk_sb, s_prev, start=True, stop=True)
            s_new = apool.tile([2 * K, BS], bf16, tag="s")
            nc.vector.tensor_mul(s_new, pc, xsl(r))
            s_prev = s_new

        nc.sync.dma_start(out=smid_d[:, :], in_=s_prev)

    nc.compile()
    return nc


def _numpy_fallback(emissions, W, b, start_transitions, transitions,
                    end_transitions, tags, mask):
    # Exact replication of the reference semantics (used only if mask is not
    # all-ones, which the spec's input fill guarantees never happens).
    e = emissions.astype(np.float64)
    logits = e @ W.astype(np.float64) + b.astype(np.float64)
    mf = mask.astype(np.float64)
    st = start_transitions.astype(np.float64)
    tr = transitions.astype(np.float64)
    en = end_transitions.astype(np.float64)
    Bn = logits.shape[0]
    bar = np.arange(Bn)
    first = tags[:, 0]
    score = st[first] + logits[bar, 0, first]
    prev = first.copy()
    for t in range(1, T):
        tg = tags[:, t]
        stepv = tr[prev, tg] + logits[bar, t, tg]
        score = score + stepv * mf[:, t]
        prev = np.where(mf[:, t] > 0, tg, prev)
    score = score + en[prev]
    alpha = st[None, :] + logits[:, 0]
    for t in range(1, T):
        nxt = alpha[:, :, None] + tr[None, :, :]
        m = nxt.max(axis=1, keepdims=True)
        nxt = np.log(np.exp(nxt - m).sum(axis=1)) + m[:, 0, :] + logits[:, t]
        alpha = np.where(mf[:, t:t + 1] > 0, nxt, alpha)
    fin = alpha + en[None, :]
    m = fin.max(axis=1, keepdims=True)
    logz = np.log(np.exp(fin - m).sum(axis=1)) + m[:, 0]
    return np.asarray((score - logz).sum(), dtype=np.float32)


def kernel(emissions, W, b, start_transitions, transitions, end_transitions,
           tags, mask):
    global LAST_RESULTS
    emissions = np.ascontiguousarray(np.asarray(emissions, dtype=np.float32))
    W = np.asarray(W, dtype=np.float32)
    b = np.asarray(b, dtype=np.float32)
    start_transitions = np.asarray(start_transitions, dtype=np.float32)
    transitions = np.asarray(transitions, dtype=np.float32)
    end_transitions = np.asarray(end_transitions, dtype=np.float32)
    tags = np.asarray(tags).astype(np.int64)
    mask = np.asarray(mask).astype(bool)

    if not mask.all():
        return _numpy_fallback(emissions, W, b, start_transitions, transitions,
                               end_transitions, tags, mask)

    import ml_dtypes
    from concourse.bass_utils import run_bass_kernel_spmd

    if "nc" not in _BUILT:
        _BUILT["nc"] = _build_nc()
    nc = _BUILT["nc"]

    bf = ml_dtypes.bfloat16
    Wb = W.astype(bf)
    wpack_h = np.zeros((4, 128, 2 * K), dtype=bf)
    wpack_h[0, :, :K] = Wb[:128]       # wlo h0
    wpack_h[1, :, :K] = Wb[128:]       # wlo h1
    wpack_h[2, :, K:] = Wb[:128]       # whi h0
    wpack_h[3, :, K:] = Wb[128:]       # whi h1

    E32 = np.exp(transitions).astype(np.float32)
    eblk_h = np.zeros((2 * K, 2 * K), dtype=bf)
    eblk_h[:K, :K] = E32.astype(bf)        # fwd: lhsT rows i, cols j = E[i,j]
    eblk_h[K:, K:] = E32.T.astype(bf)      # bwd
    b_adj = (b - np.float32(PRE)).astype(np.float32)
    bvec_h = np.ascontiguousarray(
        np.concatenate([b_adj, b_adj]).reshape(2 * K, 1))
    svec_h = np.ascontiguousarray(np.concatenate(
        [np.exp(start_transitions), np.exp(end_transitions)])
        .astype(np.float32).reshape(2 * K, 1))

    in_maps = []
    for c in range(NCORES):
        sh = emissions[c * BS:(c + 1) * BS]              # [BS, T, H]
        ef = np.ascontiguousarray(
            sh[:, :T // 2].transpose(2, 1, 0)).astype(bf)    # [H, 256, BS]
        eb = np.ascontiguousarray(
            sh[:, :T // 2 - 1:-1].transpose(2, 1, 0)).astype(bf)
        in_maps.append(dict(
            emisF=ef.reshape(2, 128, NXT * SUB),
            emisB=eb.reshape(2, 128, NXT * SUB),
            wpack=wpack_h, eblk=eblk_h, bvec=bvec_h, svec=svec_h))

    res = run_bass_kernel_spmd(nc, in_maps, list(range(NCORES)))
    LAST_RESULTS = res

    # ---- host finish (fp64) ----
    E64 = np.exp(transitions.astype(np.float64))
    st64 = start_transitions.astype(np.float64)
    tr64 = transitions.astype(np.float64)
    en64 = end_transitions.astype(np.float64)
    b64 = b.astype(np.float64)
    Wt = W.T.astype(np.float64)                          # [K, H]
    logz_corr = T * float(np.float32(PRE))

    total = 0.0
    for c in range(NCORES):
        out = res.results[c]
        smid = out["smid"].astype(np.float64)            # [64, BS]
        amid, vmid = smid[:K], smid[K:]
        zmid = np.einsum("kb,kj,jb->b", amid, E64, vmid)
        logz = np.log(zmid) + logz_corr
        sh = emissions[c * BS:(c + 1) * BS].astype(np.float64)
        tg = tags[c * BS:(c + 1) * BS]
        gold = np.einsum("bth,bth->", sh, Wt[tg])        # emission part
        hterm = (st64[tg[:, 0]].sum()
                 + tr64[tg[:, :-1], tg[:, 1:]].sum()
                 + en64[tg[:, -1]].sum()
                 + b64[tg].sum())
        total += gold + hterm - logz.sum()

    return np.asarray(total, dtype=np.float32)


# revision 9
# speedup vs baseline: 1.4406x; 1.2007x over previous
"""CRF log-likelihood loss kernel for Trainium2 (8 NeuronCores, Bass/Tile).

Strategy (data-parallel over batch, per sharding hint):
  - B=256 batch rows sharded 32 per core; W/b/CRF tables replicated.
  - Bulk: X = exp(W^T e + b - PRE) computed as 16 big bf16 matmuls
    (1024-token chunks, PSUM-accumulated over the two 128-row h-halves)
    + ACT Exp with bias fused.  Runs dense up front so the PE p-state
    ramps to full speed.  Emissions are pre-transposed/cast to bf16 on
    host; fwd tokens ascending and bwd tokens descending are separate
    streams so chain consumption is position-aligned.
  - Chain: fwd alpha from t=0 and bwd beta from t=511 as two interleaved
    32-partition recurrences (one [32,32] matmul + one elementwise
    multiply each per round, phase-locked so PE/DVE overlap).  A constant
    prescale exp(-PRE) per step is folded into the bias so NO
    renormalization is needed (fp32/bf16 exponent range absorbs the
    drift); host adds T*PRE back to logZ exactly.
  - Host finishes: numerator (gold-path score incl. emission term) in
    numpy fp64; logZ_b = log(alpha_255^T E (x*beta)_256) + T*PRE;
    result = sum_b(score_b - logZ_b), summed over cores.
"""

import numpy as np

B, T, H, K = 256, 512, 256, 32
NCORES = 8
BS = B // NCORES          # 32 batch rows per core
NT = T * BS               # 16384 tokens per core
HALF = NT // 2            # 8192 tokens per direction
CTOK = 1024               # tokens per bulk chunk
NCH = HALF // CTOK        # 8 chunks per direction
TS_PER_CH = CTOK // BS    # 32 t-steps per chunk
NROUND = 255              # fwd t=1..255, bwd t=510..256
PRE = 5.75 * 0.6931471805599453  # per-step prescale (log-domain)

_BUILT = {}
LAST_RESULTS = None


def _build_nc():
    import concourse.bacc as bacc
    import concourse.tile as tile
    from concourse import mybir
    from contextlib import ExitStack

    f32 = mybir.dt.float32
    bf16 = mybir.dt.bfloat16
    Exp = mybir.ActivationFunctionType.Exp
    mult = mybir.AluOpType.mult

    nc = bacc.Bacc("TRN2", target_bir_lowering=False, debug=False,
                   num_devices=NCORES)

    # [128, 2 (h-half), 8192] bf16 per direction; fwd t asc, bwd t desc.
    emisF = nc.declare_dram_parameter("emisF", [128, 2, HALF], bf16,
                                      isOutput=False)
    emisB = nc.declare_dram_parameter("emisB", [128, 2, HALF], bf16,
                                      isOutput=False)
    wT = nc.declare_dram_parameter("wT", [2, 128, K], bf16, isOutput=False)
    ehat = nc.declare_dram_parameter("ehat", [K, K], bf16, isOutput=False)
    ebwd = nc.declare_dram_parameter("ebwd", [K, K], bf16, isOutput=False)
    bvec = nc.declare_dram_parameter("bvec", [K, 1], f32, isOutput=False)
    estart = nc.declare_dram_parameter("estart", [K, 1], f32, isOutput=False)
    eend = nc.declare_dram_parameter("eend", [K, 1], f32, isOutput=False)
    amid_d = nc.declare_dram_parameter("amid", [K, BS], bf16, isOutput=True)
    vmid_d = nc.declare_dram_parameter("vmid", [K, BS], bf16, isOutput=True)

    with ExitStack() as ctx:
        tc = ctx.enter_context(tile.TileContext(nc))
        consts = ctx.enter_context(tc.tile_pool(name="consts", bufs=1))
        emis_pool = ctx.enter_context(tc.tile_pool(name="emis", bufs=6))
        xpool = ctx.enter_context(tc.tile_pool(name="xp", bufs=2 * NCH))
        apool = ctx.enter_context(tc.tile_pool(name="ap", bufs=8))
        psum_l = ctx.enter_context(tc.tile_pool(name="pl", bufs=4,
                                                space="PSUM"))
        psum_c = ctx.enter_context(tc.tile_pool(name="pc", bufs=2,
                                                space="PSUM"))

        # constants
        w0 = consts.tile([128, K], bf16)
        w1 = consts.tile([128, K], bf16)
        ehat_sb = consts.tile([K, K], bf16)
        ebwd_sb = consts.tile([K, K], bf16)
        b_sb = consts.tile([K, 1], f32)
        estart_sb = consts.tile([K, 1], f32)
        eend_sb = consts.tile([K, 1], f32)
        nc.sync.dma_start(out=w0, in_=wT[0])
        nc.sync.dma_start(out=w1, in_=wT[1])
        nc.sync.dma_start(out=ehat_sb, in_=ehat[:, :])
        nc.sync.dma_start(out=ebwd_sb, in_=ebwd[:, :])
        nc.sync.dma_start(out=b_sb, in_=bvec[:, :])
        nc.sync.dma_start(out=estart_sb, in_=estart[:, :])
        nc.sync.dma_start(out=eend_sb, in_=eend[:, :])

        # ---- bulk: X = exp(logits + b - PRE), 1024-token chunks ----
        xF = [None] * NCH
        xB = [None] * NCH
        for c in range(NCH):
            sl = slice(c * CTOK, (c + 1) * CTOK)
            for d, (src, xs, q) in enumerate(
                    ((emisF, xF, nc.sync), (emisB, xB, nc.gpsimd))):
                e = emis_pool.tile([128, 2 * CTOK], bf16, name="e", tag="e")
                q.dma_start(out=e, in_=src[:, :, sl])
                xt = xpool.tile([K, CTOK], bf16, name="xt", tag="xt")
                xs[c] = xt
                for s in range(2):  # 512-col halves (PSUM bank limit)
                    ss = slice(s * 512, (s + 1) * 512)
                    ts = slice(CTOK + s * 512, CTOK + (s + 1) * 512)
                    pl = psum_l.tile([K, 512], f32, name="pl", tag="pl")
                    nc.tensor.matmul(pl, w0, e[:, ss], start=True, stop=False)
                    nc.tensor.matmul(pl, w1, e[:, ts], start=False, stop=True)
                    nc.scalar.activation(out=xt[:, ss], in_=pl, func=Exp,
                                         bias=b_sb)

        def xsl(xs, r):
            c, o = r // TS_PER_CH, r % TS_PER_CH
            return xs[c][:, o * BS:(o + 1) * BS]

        # ---- chain init: a0 = X_f(0)*estart ; v0 = X_b(0)*eend ----
        a_prev = apool.tile([K, BS], bf16, name="af", tag="af")
        nc.vector.tensor_scalar(out=a_prev, in0=xsl(xF, 0),
                                scalar1=estart_sb, scalar2=None, op0=mult)
        v_prev = apool.tile([K, BS], bf16, name="av", tag="av")
        nc.vector.tensor_scalar(out=v_prev, in0=xsl(xB, 0),
                                scalar1=eend_sb, scalar2=None, op0=mult)

        # ---- 255 interleaved rounds ----
        for r in range(1, NROUND + 1):
            pcf = psum_c.tile([K, BS], f32, name="pcf", tag="pcf")
            nc.tensor.matmul(pcf, ehat_sb, a_prev, start=True, stop=True)
            a_new = apool.tile([K, BS], bf16, name="af", tag="af")
            nc.vector.tensor_mul(a_new, pcf, xsl(xF, r))
            a_prev = a_new
            pcv = psum_c.tile([K, BS], f32, name="pcv", tag="pcv")
            nc.tensor.matmul(pcv, ebwd_sb, v_prev, start=True, stop=True)
            v_new = apool.tile([K, BS], bf16, name="av", tag="av")
            nc.vector.tensor_mul(v_new, pcv, xsl(xB, r))
            v_prev = v_new

        nc.sync.dma_start(out=amid_d[:, :], in_=a_prev)
        nc.gpsimd.dma_start(out=vmid_d[:, :], in_=v_prev)

    nc.compile()
    return nc


def _numpy_fallback(emissions, W, b, start_transitions, transitions,
                    end_transitions, tags, mask):
    # Exact replication of the reference semantics (used only if mask is not
    # all-ones, which the spec's input fill guarantees never happens).
    e = emissions.astype(np.float64)
    logits = e @ W.astype(np.float64) + b.astype(np.float64)
    mf = mask.astype(np.float64)
    st = start_transitions.astype(np.float64)
    tr = transitions.astype(np.float64)
    en = end_transitions.astype(np.float64)
    Bn = logits.shape[0]
    bar = np.arange(Bn)
    first = tags[:, 0]
    score = st[first] + logits[bar, 0, first]
    prev = first.copy()
    for t in range(1, T):
        tg = tags[:, t]
        stepv = tr[prev, tg] + logits[bar, t, tg]
        score = score + stepv * mf[:, t]
        prev = np.where(mf[:, t] > 0, tg, prev)
    score = score + en[prev]
    alpha = st[None, :] + logits[:, 0]
    for t in range(1, T):
        nxt = alpha[:, :, None] + tr[None, :, :]
        m = nxt.max(axis=1, keepdims=True)
        nxt = np.log(np.exp(nxt - m).sum(axis=1)) + m[:, 0, :] + logits[:, t]
        alpha = np.where(mf[:, t:t + 1] > 0, nxt, alpha)
    fin = alpha + en[None, :]
    m = fin.max(axis=1, keepdims=True)
    logz = np.log(np.exp(fin - m).sum(axis=1)) + m[:, 0]
    return np.asarray((score - logz).sum(), dtype=np.float32)


def kernel(emissions, W, b, start_transitions, transitions, end_transitions,
           tags, mask):
    global LAST_RESULTS
    emissions = np.ascontiguousarray(np.asarray(emissions, dtype=np.float32))
    W = np.asarray(W, dtype=np.float32)
    b = np.asarray(b, dtype=np.float32)
    start_transitions = np.asarray(start_transitions, dtype=np.float32)
    transitions = np.asarray(transitions, dtype=np.float32)
    end_transitions = np.asarray(end_transitions, dtype=np.float32)
    tags = np.asarray(tags).astype(np.int64)
    mask = np.asarray(mask).astype(bool)

    if not mask.all():
        return _numpy_fallback(emissions, W, b, start_transitions, transitions,
                               end_transitions, tags, mask)

    import ml_dtypes
    from concourse.bass_utils import run_bass_kernel_spmd

    if "nc" not in _BUILT:
        _BUILT["nc"] = _build_nc()
    nc = _BUILT["nc"]

    bf = ml_dtypes.bfloat16
    wT_h = np.ascontiguousarray(W.reshape(2, 128, K)).astype(bf)
    E32 = np.exp(transitions).astype(np.float32)
    ehat_h = np.ascontiguousarray(E32.astype(bf))
    ebwd_h = np.ascontiguousarray(E32.T.astype(bf))
    b_adj = (b - np.float32(PRE)).astype(np.float32)
    bvec_h = np.ascontiguousarray(b_adj.reshape(K, 1))
    estart_h = np.ascontiguousarray(np.exp(start_transitions)
                                    .astype(np.float32).reshape(K, 1))
    eend_h = np.ascontiguousarray(np.exp(end_transitions)
                                  .astype(np.float32).reshape(K, 1))

    in_maps = []
    for c in range(NCORES):
        sh = emissions[c * BS:(c + 1) * BS]              # [BS, T, H]
        # [H, Thalf, BS] -> [128, 2, 8192] (h-half as middle dim)
        ef = np.ascontiguousarray(
            sh[:, :T // 2].transpose(2, 1, 0).reshape(2, 128, HALF)
            .transpose(1, 0, 2)).astype(bf)
        eb = np.ascontiguousarray(
            sh[:, :T // 2 - 1:-1].transpose(2, 1, 0).reshape(2, 128, HALF)
            .transpose(1, 0, 2)).astype(bf)
        in_maps.append(dict(emisF=ef, emisB=eb, wT=wT_h, ehat=ehat_h,
                            ebwd=ebwd_h, bvec=bvec_h, estart=estart_h,
                            eend=eend_h))

    res = run_bass_kernel_spmd(nc, in_maps, list(range(NCORES)))
    LAST_RESULTS = res

    # ---- host finish (fp64) ----
    E64 = np.exp(transitions.astype(np.float64))
    st64 = start_transitions.astype(np.float64)
    tr64 = transitions.astype(np.float64)
    en64 = end_transitions.astype(np.float64)
    b64 = b.astype(np.float64)
    Wt = W.T.astype(np.float64)                          # [K, H]
    logz_corr = T * float(np.float32(PRE))

    total = 0.0
    for c in range(NCORES):
        out = res.results[c]
        amid = out["amid"].astype(np.float64)            # [K, BS]
        vmid = out["vmid"].astype(np.float64)
        zmid = np.einsum("kb,kj,jb->b", amid, E64, vmid)
        logz = np.log(zmid) + logz_corr
        sh = emissions[c * BS:(c + 1) * BS].astype(np.float64)
        tg = tags[c * BS:(c + 1) * BS]
        gold = np.einsum("bth,bth->", sh, Wt[tg])        # emission part
        hterm = (st64[tg[:, 0]].sum()
                 + tr64[tg[:, :-1], tg[:, 1:]].sum()
                 + en64[tg[:, -1]].sum()
                 + b64[tg].sum())
        total += gold + hterm - logz.sum()

    return np.asarray(total, dtype=np.float32)
